# revision 91
# baseline (speedup 1.0000x reference)
"""Trainium2 Bass kernel for nn_DataReuploadingEncoder (4-qubit data
re-uploading circuit, B=1048576 samples, 8-core data parallel).

v2 layout: state real-rep in partitions: p = 32*g + 16*r + j for group
g in 0..3, r in {re,im}, state index j in 0..15; F=512 sample columns
per tile (2048 samples/tile).  Each complex 16x16 gate is TWO
PSUM-accumulated 512-col matmuls with dense 32x32 real-rep blocks:

    P = G1 . (CS o S) + G2 . (SN o S)

where CS/SN are duplicated/RE-IM-signed cos/sin diag-phase tiles and
the elementwise products run on DVE in fp16/all-SBUF (2x/4x mode).
The r-signed phases come straight out of the phase matmul stationary
(sign sigma(r) folded in), so ONE Sin activation yields the signed sin
and one Abs+Sin the (even) cos.  Layer-0's D1 acts on the uniform
state so CS1/SN1 feed the first gate directly (masked stationaries).

Emission is a modulo software pipeline over lockstep tile pairs
(L0(p) | L1(p-1) | L2(p-2) | finish(p-3..4) | trig(p+2) per step) so
every in-order engine queue interleaves ~5 pairs; the two chains of a
pair are skewed so each chain's diag-multiply round trip hides behind
the sibling's matmuls.  All diag multiplies read gate outputs straight
from PSUM on DVE (1x but one hop shorter than copy+mul) -- this keeps
the PE gap-free so it holds the high p-state (~380ns/512-col matmul
instead of ~600).  Engine split: PE gate/phase/sign matmuls; DMA XBAR
(dma_start_transpose) for both input and output transposes; ScalarE
trig Sin/Abs + final Square; DVE diag multiplies + finish casts;
GpSimd sin^2 and 1-2s^2 combos.  The x2 of sin(P)=2 sin(P/2)cos(P/2)
is folded into the G2 stationaries so SN1 is a plain fp16 multiply.
Act-table thrash is avoided by doing Tanh as 4 big flat-chunk
activations before any Sin is scheduled.

I/O: contiguous flat DMAs (4-byte strided DMA is slow).
Sample mapping per core: s(p, b, h, u2, g) = (nflat/4)*p + 32*b +
16*h + 4*u2 + g; tile t = 2b+h, phase col c = 128*u2 + p.
"""

import numpy as np

N_QUBITS = 4
N_LAYERS = 3
DIM = 16
GRP = 4        # sample groups per tile (partition packing)
F = 512        # sample columns per tile
TS = GRP * F   # samples per tile
N_CORES = 8

DT_STATE = "float16"

# ----------------------------------------------------------------------------
# host-side constant construction
# ----------------------------------------------------------------------------


def _rz(t):
    return np.diag([np.exp(-0.5j * t), np.exp(0.5j * t)]).astype(np.complex128)


def _ry(t):
    c, s = np.cos(t / 2), np.sin(t / 2)
    return np.array([[c, -s], [s, c]], dtype=np.complex128)


def _rot(phi, theta, omega):
    return _rz(omega) @ _ry(theta) @ _rz(phi)


def _kron4(mats):
    out = mats[0]
    for m in mats[1:]:
        out = np.kron(out, m)
    return out


def _cnot_mat(c, t):
    P = np.zeros((DIM, DIM), dtype=np.complex128)
    for j in range(DIM):
        bc = (j >> (3 - c)) & 1
        jj = j ^ (1 << (3 - t)) if bc else j
        P[jj, j] = 1.0
    return P


def _bit(j, i):
    return (j >> (3 - i)) & 1


def _host_tensors(weights, scaling, dt_state=np.float16):
    weights = np.asarray(weights, dtype=np.float64)
    scaling = np.asarray(scaling, dtype=np.float64)
    dt = dt_state

    # A_half[l, i, j] = sgn_ij * scaling[l, i] / 2   (pi folded in act scale)
    A = np.zeros((N_LAYERS, N_QUBITS, DIM))
    for l in range(N_LAYERS):
        for i in range(N_QUBITS):
            for j in range(DIM):
                sgn = 1.0 if _bit(j, i) else -1.0
                A[l, i, j] = sgn * scaling[l, i] / 2.0
    uniq, lmap = [], []
    for l in range(N_LAYERS):
        for k, ku in enumerate(uniq):
            if np.array_equal(A[l], A[ku]):
                lmap.append(k)
                break
        else:
            uniq.append(l)
            lmap.append(len(uniq) - 1)
    A_u = A[uniq]
    nu = len(uniq)

    # phase stationaries, partition-dim first:
    # phm[k=(4m'+i), lu, u, m=(32g+16r+j)] = sigma(r) A_u[lu,i,j],
    #   m' = 16h + 4u2 + g,  u = 4h + u2
    phm = np.zeros((128, nu, 8, 128), dtype=np.float64)
    for lu in range(nu):
        for u in range(8):
            h, u2 = divmod(u, 4)
            for g in range(GRP):
                mp = 16 * h + 4 * u2 + g
                for i in range(N_QUBITS):
                    for r in range(2):
                        sig = -1.0 if r == 0 else 1.0
                        phm[4 * mp + i, lu, u, 32 * g + 16 * r:
                            32 * g + 16 * r + 16] = sig * A_u[lu, i]

    S = np.diag([1.0, 1.0j]).astype(np.complex128)
    H = np.array([[1, 1], [1, -1]], dtype=np.complex128) / np.sqrt(2.0)
    SH = S @ H
    HSd = H @ S.conj().T
    C = np.eye(DIM, dtype=np.complex128)
    for i in range(N_QUBITS):
        C = _cnot_mat(i, (i + 1) % N_QUBITS) @ C
    F_SH = _kron4([SH] * 4)
    F_HS = _kron4([HSd] * 4)
    R = [_kron4([_rot(*weights[l, i]) for i in range(N_QUBITS)])
         for l in range(N_LAYERS)]
    W_T2 = [F_HS @ C @ R[0], F_HS @ C @ R[1], C @ R[2]]

    def g1(W):
        Wr, Wi = np.real(W), np.imag(W)
        return np.block([[Wr, -Wi], [Wi, Wr]])

    def g2(W):
        Wr, Wi = np.real(W), np.imag(W)
        return np.block([[Wi, -Wr], [-Wr, -Wi]])

    Z = np.zeros((DIM, DIM))

    def f0(W):  # l=0 T1: ma = CS1 (dup cos), mb = SN1 (signed sin)
        Wr, Wi = np.real(W), np.imag(W)
        return np.block([[Wr, Z], [Wi, Z]])

    def f2_0(W):
        Wr, Wi = np.real(W), np.imag(W)
        return np.block([[Z, -Wi], [Z, Wr]])

    # the SN1 tile on-device is sin(P/2)cos(P/2) = sin(P)/2: fold the
    # missing factor 2 into the stationaries that consume it
    W0 = 0.25 * F_SH
    gate_mats = [f0(W0), 2.0 * f2_0(W0), g1(F_SH), 2.0 * g2(F_SH)]
    for l in range(N_LAYERS):
        gate_mats += [g1(W_T2[l]), g2(W_T2[l])]

    # lhsT, partition-dim first, block-diag over 4 groups, transposed blocks
    gm = np.zeros((128, len(gate_mats), 128), dtype=np.float64)
    for gi, G in enumerate(gate_mats):
        for g in range(GRP):
            gm[32 * g:32 * g + 32, gi, 32 * g:32 * g + 32] = G.T

    # two sign stationaries: chain c writes rows 16c..16c+16 of a shared
    # [32, F] PSUM tile (the other half's columns are zero)
    sg = np.zeros((2, 128, 32), dtype=np.float64)
    for c in range(2):
        for g in range(GRP):
            for r in range(2):
                for j in range(DIM):
                    for w in range(N_QUBITS):
                        sg[c, 32 * g + 16 * r + j, 16 * c + 4 * g + w] = \
                            1.0 - 2.0 * _bit(j, w)

    id128 = np.eye(128)
    id32 = np.eye(32)

    consts = {"phmats": phm.astype(np.float32).astype(dt),
              "gmats": gm.astype(np.float32).astype(dt),
              "sgmat": sg.astype(dt),
              "id128": id128.astype(dt),
              "id32": id32.astype(dt)}
    return consts, nu, lmap


# ----------------------------------------------------------------------------
# bass kernel
# ----------------------------------------------------------------------------

_NC_CACHE = {}


def _build_nc(bs, nu, lmap, dt_state_name=None):
    import concourse.tile as tile
    from concourse import bacc, mybir
    from contextlib import ExitStack

    f32 = mybir.dt.float32
    dt_st = getattr(mybir.dt, dt_state_name or DT_STATE)
    ACT = mybir.ActivationFunctionType
    MULT = mybir.AluOpType.mult
    ADD = mybir.AluOpType.add
    PIH = float(np.pi / 2)

    assert bs % TS == 0
    ntiles = bs // TS
    nflat = bs * N_QUBITS // 128
    nblk = nflat // 128
    assert ntiles == 2 * nblk
    NG = 4 + 2 * N_LAYERS

    nc = bacc.Bacc("TRN2", target_bir_lowering=False, debug=False)
    x_ap = nc.dram_tensor("x", [bs, N_QUBITS], f32, kind="ExternalInput").ap()
    phm_ap = nc.dram_tensor("phmats", [128, nu, 8, 128], dt_st,
                            kind="ExternalInput").ap()
    gm_ap = nc.dram_tensor("gmats", [128, NG, 128], dt_st,
                           kind="ExternalInput").ap()
    sg_ap = nc.dram_tensor("sgmat", [2, 128, 32], dt_st,
                           kind="ExternalInput").ap()
    id128_ap = nc.dram_tensor("id128", [128, 128], dt_st,
                              kind="ExternalInput").ap()
    id32_ap = nc.dram_tensor("id32", [32, 32], dt_st,
                             kind="ExternalInput").ap()
    out_ap = nc.dram_tensor("out", [bs, N_QUBITS], f32,
                            kind="ExternalOutput").ap()

    halfpi = nc.alloc_sbuf_tensor("halfpi", [128, 1], f32)
    nc.gpsimd.memset(halfpi.ap(), PIH)
    nc.all_engine_barrier()

    with tile.TileContext(nc) as tc:
        with ExitStack() as ctx:
            consts = ctx.enter_context(tc.tile_pool(name="consts", bufs=1))
            bigp = ctx.enter_context(tc.tile_pool(name="big", bufs=1))
            trig = ctx.enter_context(tc.tile_pool(name="trig", bufs=10))
            mp = ctx.enter_context(tc.tile_pool(name="mp", bufs=3))
            sqp = ctx.enter_context(tc.tile_pool(name="sq", bufs=3))
            phip = ctx.enter_context(tc.tile_pool(name="phip", bufs=1,
                                                  space="PSUM"))
            gp = ctx.enter_context(tc.tile_pool(name="gp", bufs=5,
                                                space="PSUM"))
            op = ctx.enter_context(tc.tile_pool(name="op", bufs=1,
                                                space="PSUM"))

            # constants
            phm = consts.tile([128, nu, 8, 128], dt_st)
            nc.sync.dma_start(phm[:], phm_ap[:])
            gm = consts.tile([128, NG, 128], dt_st)
            nc.sync.dma_start(gm[:], gm_ap[:])
            sgm = consts.tile([128, 2, 32], dt_st)
            for c in range(2):
                nc.sync.dma_start(sgm[:, c, :], sg_ap[c, :, :])
            id128 = consts.tile([128, 128], dt_st)
            nc.sync.dma_start(id128[:], id128_ap[:])
            id32 = consts.tile([32, 32], dt_st)
            nc.sync.dma_start(id32[:], id32_ap[:])

            # ---- prologue: load, tanh (flat, 4 big acts), transpose -------
            fl = bigp.tile([128, nflat], f32)
            thf = bigp.tile([128, nflat], dt_st)
            xflat = x_ap[:].rearrange("(p s) i -> p (s i)", p=128)
            nchunk = 4
            cw = nflat // nchunk
            for c in range(nchunk):
                nc.sync.dma_start(fl[:, c * cw:(c + 1) * cw],
                                  xflat[:, c * cw:(c + 1) * cw])
                nc.scalar.activation(thf[:, c * cw:(c + 1) * cw],
                                     fl[:, c * cw:(c + 1) * cw], ACT.Tanh)
            th = bigp.tile([128, nblk, 128], dt_st)
            for b in range(nblk):
                nc.sync.dma_start_transpose(th[:, b, :],
                                            thf[:, 128 * b:128 * (b + 1)])

            od = bigp.tile([128, ntiles, 4, 16], f32)

            def emit_phase_super(s, h):
                """Phase mms for parity h of pairs 2s and 2s+1: 4 mms of
                256 cols each (two th blocks per moving)."""
                phi2 = phip.tile([128, nu, 4, 2, 128], f32, tag="phi2")
                for lu in range(nu):
                    for u2 in range(4):
                        nc.tensor.matmul(
                            phi2[:, lu, u2, :, :],
                            phm[:, lu, 4 * h + u2, :],
                            th[:, 2 * s:2 * s + 2, :],
                            start=True, stop=True)
                return phi2

            def emit_trig(t, phi2):
                bb = (t // 2) % 2
                csn1s, csn2s = [], []
                for lu in range(nu):
                    phif = phi2[:, lu, :, bb, :]        # [128, 4, 128]
                    aphi = trig.tile([128, 4, 128], f32, tag=f"aphi{lu}")
                    nc.scalar.activation(aphi[:], phif, ACT.Abs)
                    csn2 = trig.tile([128, 2, F], dt_st, tag=f"csn2_{lu}")
                    nc.scalar.activation(
                        csn2[:, 1, :].rearrange("p (a b) -> p a b", a=4),
                        phif, ACT.Sin, scale=PIH)
                    nc.scalar.activation(
                        csn2[:, 0, :].rearrange("p (a b) -> p a b", a=4),
                        aphi[:], ACT.Sin, scale=-PIH, bias=halfpi.ap())
                    ssq = trig.tile([128, F], dt_st, tag=f"ssq{lu}")
                    nc.gpsimd.tensor_tensor(out=ssq[:], in0=csn2[:, 1, :],
                                            in1=csn2[:, 1, :], op=MULT)
                    csn1 = trig.tile([128, 2, F], dt_st, tag=f"csn1_{lu}")
                    nc.gpsimd.tensor_scalar(out=csn1[:, 0, :], in0=ssq[:],
                                            scalar1=-2.0, scalar2=1.0,
                                            op0=MULT, op1=ADD)
                    nc.vector.tensor_tensor(out=csn1[:, 1, :],
                                            in0=csn2[:, 1, :],
                                            in1=csn2[:, 0, :], op=MULT)
                    csn1s.append(csn1)
                    csn2s.append(csn2)
                return csn1s, csn2s

            def emit_phase_trig_super(s):
                """Phases+trig for pairs 2s, 2s+1; returns 2 trig-pair sets."""
                out = []
                for h in range(2):
                    phi2 = emit_phase_super(s, h)
                    # tiles with parity h of both pairs consume this phi2
                    out.append([emit_trig(2 * (2 * s + pp) + h, phi2)
                                for pp in range(2)])
                # out[h][pp] -> regroup to per-pair [pp][c=h]
                return [[out[0][pp], out[1][pp]] for pp in range(2)]

            def dmul(dst_tag, src, csn, eng=None):
                """m[:, q, :] = src o csn[:, q, :]  (fused pair).  src may be
                an SBUF fp16 state copy or a PSUM fp32 gate output."""
                m = mp.tile([128, 2, F], dt_st, tag=dst_tag)
                (eng or nc.vector).tensor_tensor(
                    out=m[:],
                    in0=src[:].unsqueeze(1).to_broadcast((128, 2, F)),
                    in1=csn[:],
                    op=MULT)
                return m

            def cgate(gi, ma, mb, tag):
                P = gp.tile([128, F], f32, tag="P", name=tag)
                nc.tensor.matmul(P[:], gm[:, gi, :], ma, start=True,
                                 stop=False)
                nc.tensor.matmul(P[:], gm[:, gi + 1, :], mb, start=False,
                                 stop=True)
                return P

            # ---- main loop: CH tiles in lockstep, trig one pair ahead ----
            CH = 2
            npairs = ntiles // CH
            assert ntiles % CH == 0

            def stage_L(p, l, st):
                """Layer l (both chains) of pair p: gate, copy, diag-mul,
                gate, next-layer diag-mul prep."""
                trigs = st["trigs"]
                k = lmap[l]
                P1s = [None] * CH
                m2s = [None] * CH

                def t1(c):
                    csn1s, _ = trigs[c]
                    if l == 0:
                        P1s[c] = cgate(0, csn1s[k][:, 0, :],
                                       csn1s[k][:, 1, :], f"P1{c}")
                    else:
                        m1 = st["m1s"][c]
                        P1s[c] = cgate(2, m1[:, 0, :], m1[:, 1, :],
                                       f"P1{c}")

                def mid(c):
                    # diag-multiply straight off PSUM: one hop shorter than
                    # copy+mul, keeps the PE fed sooner
                    m2s[c] = dmul(f"m2_{l}{c}", P1s[c], trigs[c][1][k])

                def t2(c):
                    st["P2s"][c] = cgate(4 + 2 * l, m2s[c][:, 0, :],
                                         m2s[c][:, 1, :], f"P2{c}")

                def post(c):
                    if l < N_LAYERS - 1:
                        kn = lmap[l + 1]
                        st["m1s"][c] = dmul(f"m1_{l + 1}{c}",
                                            st["P2s"][c],
                                            trigs[c][0][kn])
                    else:
                        sq = sqp.tile([128, F], dt_st, tag=f"sq{c}",
                                      name=f"sq{c}")
                        nc.scalar.activation(sq[:], st["P2s"][c][:],
                                             ACT.Square)
                        st["sqs"][c] = sq

                # skewed: each chain's Sc/DVE round trip is emitted right
                # after its own matmuls so it hides behind the sibling's
                t1(0); mid(0); t1(1); mid(1)
                t2(0); post(0); t2(1); post(1)

            def stage_E(p, st):
                outp = op.tile([32, F], f32, tag="outp")
                for c in range(CH):
                    nc.tensor.matmul(outp[:], sgm[:, c, :], st["sqs"][c][:],
                                     start=(c == 0), stop=(c == CH - 1))
                ocp = sqp.tile([32, F], dt_st, tag="ocp")
                nc.vector.tensor_copy(out=ocp[:], in_=outp[:])
                otr = sqp.tile([128, 4, 32], dt_st, tag="otr")
                nc.sync.dma_start_transpose(otr[:], ocp[:])
                st["otr"] = otr

            def stage_E2(p, st):
                for c in range(CH):
                    nc.vector.tensor_copy(
                        out=od[:, p * CH + c, :, :],
                        in_=st["otr"][:, :, 16 * c:16 * (c + 1)])

            # modulo software pipeline over pairs: each emission step carries
            # L0(p), L1(p-1), L2(p-2), E(p-3) and trig for pair p+2 so every
            # in-order engine queue interleaves ~5 pairs' work.
            def mkstate(trigs):
                return {"trigs": trigs, "m1s": [None] * CH,
                        "P2s": [None] * CH, "sqs": [None] * CH}

            assert npairs % 2 == 0
            states = {}
            tp0 = emit_phase_trig_super(0)
            states[0] = mkstate(tp0[0])
            states[1] = mkstate(tp0[1])
            for step in range(npairs + 4):
                if step < npairs:
                    stage_L(step, 0, states[step])
                if 0 <= step - 1 < npairs:
                    stage_L(step - 1, 1, states[step - 1])
                if 0 <= step - 2 < npairs:
                    stage_L(step - 2, 2, states[step - 2])
                if 0 <= step - 4 < npairs:
                    stage_E2(step - 4, states[step - 4])
                    del states[step - 4]
                if 0 <= step - 3 < npairs:
                    stage_E(step - 3, states[step - 3])
                if step + 2 < npairs and (step + 2) % 2 == 0:
                    tps = emit_phase_trig_super((step + 2) // 2)
                    states[step + 2] = mkstate(tps[0])
                    states[step + 3] = mkstate(tps[1])

            # ---- final store (chunked for overlap) ----------------------
            oflat = out_ap[:].rearrange("(p s) w -> p (s w)", p=128)
            tchunk = ntiles // 4
            for c in range(4):
                nc.sync.dma_start(
                    oflat[:, 64 * tchunk * c:64 * tchunk * (c + 1)],
                    od[:, tchunk * c:tchunk * (c + 1), :, :]
                    .rearrange("p a b c -> p (a b c)"))

    nc.compile()
    return nc


def _get_nc(bs, nu, lmap, dt_state_name=None):
    key = (bs, nu, tuple(lmap), dt_state_name or DT_STATE)
    if key not in _NC_CACHE:
        _NC_CACHE[key] = _build_nc(bs, nu, lmap, dt_state_name)
    return _NC_CACHE[key]


def _np_dt(name):
    import ml_dtypes
    return {"float32": np.float32, "float32r": np.float32,
            "float16": np.float16, "bfloat16": ml_dtypes.bfloat16}[name]


def kernel(x, weights, scaling):
    from concourse.bass_utils import run_bass_kernel_spmd

    x = np.ascontiguousarray(np.asarray(x, dtype=np.float32))
    B = x.shape[0]
    consts, nu, lmap = _host_tensors(weights, scaling,
                                     dt_state=_np_dt(DT_STATE))

    chunk = N_CORES * TS * 2
    Bp = ((B + chunk - 1) // chunk) * chunk
    if Bp != B:
        xp = np.zeros((Bp, x.shape[1]), dtype=np.float32)
        xp[:B] = x
        x = xp
    bs = Bp // N_CORES

    nc = _get_nc(bs, nu, lmap)
    xs = x.reshape(N_CORES, bs, x.shape[1])
    in_maps = [dict(consts, x=np.ascontiguousarray(xs[i]))
               for i in range(N_CORES)]
    res = run_bass_kernel_spmd(nc, in_maps, core_ids=list(range(N_CORES)))
    out = np.concatenate([r["out"] for r in res.results], axis=0)
    return out[:B]


# revision 92
# speedup vs baseline: 1.0103x; 1.0103x over previous
"""Trainium2 Bass kernel for nn_DataReuploadingEncoder (4-qubit data
re-uploading circuit, B=1048576 samples, 8-core data parallel).

v2 layout: state real-rep in partitions: p = 32*g + 16*r + j for group
g in 0..3, r in {re,im}, state index j in 0..15; F=512 sample columns
per tile (2048 samples/tile).  Each complex 16x16 gate is TWO
PSUM-accumulated 512-col matmuls with dense 32x32 real-rep blocks:

    P = G1 . (CS o S) + G2 . (SN o S)

where CS/SN are duplicated/RE-IM-signed cos/sin diag-phase tiles and
the elementwise products run on DVE in fp16/all-SBUF (2x/4x mode).
The r-signed phases come straight out of the phase matmul stationary
(sign sigma(r) folded in), so ONE Sin activation yields the signed sin
and one Abs+Sin the (even) cos.  Layer-0's D1 acts on the uniform
state so CS1/SN1 feed the first gate directly (masked stationaries).

Emission is a modulo software pipeline over lockstep tile pairs
(L0(p) | L1(p-1) | L2(p-2) | finish(p-3..4) | trig(p+2) per step) so
every in-order engine queue interleaves ~5 pairs; the two chains of a
pair are skewed so each chain's diag-multiply round trip hides behind
the sibling's matmuls.  All diag multiplies read gate outputs straight
from PSUM on DVE (1x but one hop shorter than copy+mul) -- this keeps
the PE gap-free so it holds the high p-state (~380ns/512-col matmul
instead of ~600).  Engine split: PE gate/phase/sign matmuls; DMA XBAR
(dma_start_transpose) for both input and output transposes; ScalarE
trig Sin/Abs + final Square; DVE diag multiplies + finish casts;
GpSimd sin^2 and 1-2s^2 combos.  The x2 of sin(P)=2 sin(P/2)cos(P/2)
is folded into the G2 stationaries so SN1 is a plain fp16 multiply.
Act-table thrash is avoided by doing Tanh as 4 big flat-chunk
activations before any Sin is scheduled.

I/O: contiguous flat DMAs (4-byte strided DMA is slow).
Sample mapping per core: s(p, b, h, u2, g) = (nflat/4)*p + 32*b +
16*h + 4*u2 + g; tile t = 2b+h, phase col c = 128*u2 + p.
"""

import numpy as np

N_QUBITS = 4
N_LAYERS = 3
DIM = 16
GRP = 4        # sample groups per tile (partition packing)
F = 512        # sample columns per tile
TS = GRP * F   # samples per tile
N_CORES = 8

DT_STATE = "float16"

# ----------------------------------------------------------------------------
# host-side constant construction
# ----------------------------------------------------------------------------


def _rz(t):
    return np.diag([np.exp(-0.5j * t), np.exp(0.5j * t)]).astype(np.complex128)


def _ry(t):
    c, s = np.cos(t / 2), np.sin(t / 2)
    return np.array([[c, -s], [s, c]], dtype=np.complex128)


def _rot(phi, theta, omega):
    return _rz(omega) @ _ry(theta) @ _rz(phi)


def _kron4(mats):
    out = mats[0]
    for m in mats[1:]:
        out = np.kron(out, m)
    return out


def _cnot_mat(c, t):
    P = np.zeros((DIM, DIM), dtype=np.complex128)
    for j in range(DIM):
        bc = (j >> (3 - c)) & 1
        jj = j ^ (1 << (3 - t)) if bc else j
        P[jj, j] = 1.0
    return P


def _bit(j, i):
    return (j >> (3 - i)) & 1


def _host_tensors(weights, scaling, dt_state=np.float16):
    weights = np.asarray(weights, dtype=np.float64)
    scaling = np.asarray(scaling, dtype=np.float64)
    dt = dt_state

    # A_half[l, i, j] = sgn_ij * scaling[l, i] / 2   (pi folded in act scale)
    A = np.zeros((N_LAYERS, N_QUBITS, DIM))
    for l in range(N_LAYERS):
        for i in range(N_QUBITS):
            for j in range(DIM):
                sgn = 1.0 if _bit(j, i) else -1.0
                A[l, i, j] = sgn * scaling[l, i] / 2.0
    uniq, lmap = [], []
    for l in range(N_LAYERS):
        for k, ku in enumerate(uniq):
            if np.array_equal(A[l], A[ku]):
                lmap.append(k)
                break
        else:
            uniq.append(l)
            lmap.append(len(uniq) - 1)
    A_u = A[uniq]
    nu = len(uniq)

    # phase stationaries, partition-dim first:
    # phm[k=(4m'+i), lu, u, m=(32g+16r+j)] = sigma(r) A_u[lu,i,j],
    #   m' = 16h + 4u2 + g,  u = 4h + u2
    phm = np.zeros((128, nu, 8, 128), dtype=np.float64)
    for lu in range(nu):
        for u in range(8):
            h, u2 = divmod(u, 4)
            for g in range(GRP):
                mp = 16 * h + 4 * u2 + g
                for i in range(N_QUBITS):
                    for r in range(2):
                        sig = -1.0 if r == 0 else 1.0
                        phm[4 * mp + i, lu, u, 32 * g + 16 * r:
                            32 * g + 16 * r + 16] = sig * A_u[lu, i]

    S = np.diag([1.0, 1.0j]).astype(np.complex128)
    H = np.array([[1, 1], [1, -1]], dtype=np.complex128) / np.sqrt(2.0)
    SH = S @ H
    HSd = H @ S.conj().T
    C = np.eye(DIM, dtype=np.complex128)
    for i in range(N_QUBITS):
        C = _cnot_mat(i, (i + 1) % N_QUBITS) @ C
    F_SH = _kron4([SH] * 4)
    F_HS = _kron4([HSd] * 4)
    R = [_kron4([_rot(*weights[l, i]) for i in range(N_QUBITS)])
         for l in range(N_LAYERS)]
    W_T2 = [F_HS @ C @ R[0], F_HS @ C @ R[1], C @ R[2]]

    def g1(W):
        Wr, Wi = np.real(W), np.imag(W)
        return np.block([[Wr, -Wi], [Wi, Wr]])

    def g2(W):
        Wr, Wi = np.real(W), np.imag(W)
        return np.block([[Wi, -Wr], [-Wr, -Wi]])

    Z = np.zeros((DIM, DIM))

    def f0(W):  # l=0 T1: ma = CS1 (dup cos), mb = SN1 (signed sin)
        Wr, Wi = np.real(W), np.imag(W)
        return np.block([[Wr, Z], [Wi, Z]])

    def f2_0(W):
        Wr, Wi = np.real(W), np.imag(W)
        return np.block([[Z, -Wi], [Z, Wr]])

    # the SN1 tile on-device is sin(P/2)cos(P/2) = sin(P)/2: fold the
    # missing factor 2 into the stationaries that consume it
    W0 = 0.25 * F_SH
    gate_mats = [f0(W0), 2.0 * f2_0(W0), g1(F_SH), 2.0 * g2(F_SH)]
    for l in range(N_LAYERS):
        gate_mats += [g1(W_T2[l]), g2(W_T2[l])]

    # lhsT, partition-dim first, block-diag over 4 groups, transposed blocks
    gm = np.zeros((128, len(gate_mats), 128), dtype=np.float64)
    for gi, G in enumerate(gate_mats):
        for g in range(GRP):
            gm[32 * g:32 * g + 32, gi, 32 * g:32 * g + 32] = G.T

    # two sign stationaries: chain c writes rows 16c..16c+16 of a shared
    # [32, F] PSUM tile (the other half's columns are zero)
    sg = np.zeros((2, 128, 32), dtype=np.float64)
    for c in range(2):
        for g in range(GRP):
            for r in range(2):
                for j in range(DIM):
                    for w in range(N_QUBITS):
                        sg[c, 32 * g + 16 * r + j, 16 * c + 4 * g + w] = \
                            1.0 - 2.0 * _bit(j, w)

    id128 = np.eye(128)
    id32 = np.eye(32)

    consts = {"phmats": phm.astype(np.float32).astype(dt),
              "gmats": gm.astype(np.float32).astype(dt),
              "sgmat": sg.astype(dt),
              "id128": id128.astype(dt),
              "id32": id32.astype(dt)}
    return consts, nu, lmap


# ----------------------------------------------------------------------------
# bass kernel
# ----------------------------------------------------------------------------

_NC_CACHE = {}


def _build_nc(bs, nu, lmap, dt_state_name=None):
    import concourse.tile as tile
    from concourse import bacc, mybir
    from contextlib import ExitStack

    f32 = mybir.dt.float32
    dt_st = getattr(mybir.dt, dt_state_name or DT_STATE)
    ACT = mybir.ActivationFunctionType
    MULT = mybir.AluOpType.mult
    ADD = mybir.AluOpType.add
    PIH = float(np.pi / 2)

    assert bs % TS == 0
    ntiles = bs // TS
    nflat = bs * N_QUBITS // 128
    nblk = nflat // 128
    assert ntiles == 2 * nblk
    NG = 4 + 2 * N_LAYERS

    nc = bacc.Bacc("TRN2", target_bir_lowering=False, debug=False)
    x_ap = nc.dram_tensor("x", [bs, N_QUBITS], f32, kind="ExternalInput").ap()
    phm_ap = nc.dram_tensor("phmats", [128, nu, 8, 128], dt_st,
                            kind="ExternalInput").ap()
    gm_ap = nc.dram_tensor("gmats", [128, NG, 128], dt_st,
                           kind="ExternalInput").ap()
    sg_ap = nc.dram_tensor("sgmat", [2, 128, 32], dt_st,
                           kind="ExternalInput").ap()
    id128_ap = nc.dram_tensor("id128", [128, 128], dt_st,
                              kind="ExternalInput").ap()
    id32_ap = nc.dram_tensor("id32", [32, 32], dt_st,
                             kind="ExternalInput").ap()
    out_ap = nc.dram_tensor("out", [bs, N_QUBITS], f32,
                            kind="ExternalOutput").ap()

    halfpi = nc.alloc_sbuf_tensor("halfpi", [128, 1], f32)
    nc.gpsimd.memset(halfpi.ap(), PIH)
    nc.all_engine_barrier()

    with tile.TileContext(nc) as tc:
        with ExitStack() as ctx:
            consts = ctx.enter_context(tc.tile_pool(name="consts", bufs=1))
            bigp = ctx.enter_context(tc.tile_pool(name="big", bufs=1))
            trig = ctx.enter_context(tc.tile_pool(name="trig", bufs=10))
            mp = ctx.enter_context(tc.tile_pool(name="mp", bufs=3))
            sqp = ctx.enter_context(tc.tile_pool(name="sq", bufs=3))
            phip = ctx.enter_context(tc.tile_pool(name="phip", bufs=1,
                                                  space="PSUM"))
            gp = ctx.enter_context(tc.tile_pool(name="gp", bufs=6,
                                                space="PSUM"))
            op = ctx.enter_context(tc.tile_pool(name="op", bufs=1,
                                                space="PSUM"))

            # constants
            phm = consts.tile([128, nu, 8, 128], dt_st)
            nc.sync.dma_start(phm[:], phm_ap[:])
            gm = consts.tile([128, NG, 128], dt_st)
            nc.sync.dma_start(gm[:], gm_ap[:])
            sgm = consts.tile([128, 2, 32], dt_st)
            for c in range(2):
                nc.sync.dma_start(sgm[:, c, :], sg_ap[c, :, :])
            id128 = consts.tile([128, 128], dt_st)
            nc.sync.dma_start(id128[:], id128_ap[:])
            id32 = consts.tile([32, 32], dt_st)
            nc.sync.dma_start(id32[:], id32_ap[:])

            # ---- prologue: load, tanh (flat, 4 big acts), transpose -------
            fl = bigp.tile([128, nflat], f32)
            thf = bigp.tile([128, nflat], dt_st)
            xflat = x_ap[:].rearrange("(p s) i -> p (s i)", p=128)
            nchunk = 4
            cw = nflat // nchunk
            for c in range(nchunk):
                nc.sync.dma_start(fl[:, c * cw:(c + 1) * cw],
                                  xflat[:, c * cw:(c + 1) * cw])
                nc.scalar.activation(thf[:, c * cw:(c + 1) * cw],
                                     fl[:, c * cw:(c + 1) * cw], ACT.Tanh)
            th = bigp.tile([128, nblk, 128], dt_st)
            for b in range(nblk):
                nc.sync.dma_start_transpose(th[:, b, :],
                                            thf[:, 128 * b:128 * (b + 1)])

            od = bigp.tile([128, ntiles, 4, 16], f32)

            def emit_phase_trig(t):
                b, h = divmod(t, 2)
                phi = phip.tile([128, nu, F], f32, tag="phi")
                for lu in range(nu):
                    for u2 in range(4):
                        nc.tensor.matmul(
                            phi[:, lu, 128 * u2:128 * (u2 + 1)],
                            phm[:, lu, 4 * h + u2, :],
                            th[:, b, :], start=True, stop=True)
                csn1s, csn2s = [], []
                for lu in range(nu):
                    aphi = trig.tile([128, F], f32, tag=f"aphi{lu}")
                    phif = phi[:, lu, :]
                    nc.scalar.activation(aphi[:], phif, ACT.Abs)
                    csn2 = trig.tile([128, 2, F], dt_st, tag=f"csn2_{lu}")
                    nc.scalar.activation(csn2[:, 1, :], phif,
                                         ACT.Sin, scale=PIH)
                    nc.scalar.activation(csn2[:, 0, :], aphi[:], ACT.Sin,
                                         scale=-PIH, bias=halfpi.ap())
                    ssq = trig.tile([128, F], dt_st, tag=f"ssq{lu}")
                    nc.gpsimd.tensor_tensor(out=ssq[:], in0=csn2[:, 1, :],
                                            in1=csn2[:, 1, :], op=MULT)
                    csn1 = trig.tile([128, 2, F], dt_st, tag=f"csn1_{lu}")
                    nc.gpsimd.tensor_scalar(out=csn1[:, 0, :], in0=ssq[:],
                                            scalar1=-2.0, scalar2=1.0,
                                            op0=MULT, op1=ADD)
                    nc.vector.tensor_tensor(out=csn1[:, 1, :],
                                            in0=csn2[:, 1, :],
                                            in1=csn2[:, 0, :], op=MULT)
                    csn1s.append(csn1)
                    csn2s.append(csn2)
                return csn1s, csn2s

            def dmul(dst_tag, src, csn, eng=None):
                """m[:, q, :] = src o csn[:, q, :]  (fused pair).  src may be
                an SBUF fp16 state copy or a PSUM fp32 gate output."""
                m = mp.tile([128, 2, F], dt_st, tag=dst_tag)
                (eng or nc.vector).tensor_tensor(
                    out=m[:],
                    in0=src[:].unsqueeze(1).to_broadcast((128, 2, F)),
                    in1=csn[:],
                    op=MULT)
                return m

            def cgate(gi, ma, mb, tag):
                P = gp.tile([128, F], f32, tag="P", name=tag)
                nc.tensor.matmul(P[:], gm[:, gi, :], ma, start=True,
                                 stop=False)
                nc.tensor.matmul(P[:], gm[:, gi + 1, :], mb, start=False,
                                 stop=True)
                return P

            # ---- main loop: CH tiles in lockstep, trig one pair ahead ----
            CH = 2
            npairs = ntiles // CH
            assert ntiles % CH == 0

            def stage_L(p, l, st):
                """Layer l (both chains) of pair p: gate, copy, diag-mul,
                gate, next-layer diag-mul prep."""
                trigs = st["trigs"]
                k = lmap[l]
                P1s = [None] * CH
                m2s = [None] * CH

                def t1(c):
                    csn1s, _ = trigs[c]
                    if l == 0:
                        P1s[c] = cgate(0, csn1s[k][:, 0, :],
                                       csn1s[k][:, 1, :], f"P1{c}")
                    else:
                        m1 = st["m1s"][c]
                        P1s[c] = cgate(2, m1[:, 0, :], m1[:, 1, :],
                                       f"P1{c}")

                def mid(c):
                    # diag-multiply straight off PSUM: one hop shorter than
                    # copy+mul, keeps the PE fed sooner
                    m2s[c] = dmul(f"m2_{l}{c}", P1s[c], trigs[c][1][k])

                def t2(c):
                    st["P2s"][c] = cgate(4 + 2 * l, m2s[c][:, 0, :],
                                         m2s[c][:, 1, :], f"P2{c}")

                def post(c):
                    if l < N_LAYERS - 1:
                        kn = lmap[l + 1]
                        st["m1s"][c] = dmul(f"m1_{l + 1}{c}",
                                            st["P2s"][c],
                                            trigs[c][0][kn])
                    else:
                        sq = sqp.tile([128, F], dt_st, tag=f"sq{c}",
                                      name=f"sq{c}")
                        nc.scalar.activation(sq[:], st["P2s"][c][:],
                                             ACT.Square)
                        st["sqs"][c] = sq

                # skewed: each chain's Sc/DVE round trip is emitted right
                # after its own matmuls so it hides behind the sibling's
                t1(0); mid(0); t1(1); mid(1)
                t2(0); post(0); t2(1); post(1)

            def stage_E(p, st):
                outp = op.tile([32, F], f32, tag="outp")
                for c in range(CH):
                    nc.tensor.matmul(outp[:], sgm[:, c, :], st["sqs"][c][:],
                                     start=(c == 0), stop=(c == CH - 1))
                ocp = sqp.tile([32, F], dt_st, tag="ocp")
                nc.vector.tensor_copy(out=ocp[:], in_=outp[:])
                otr = sqp.tile([128, 4, 32], dt_st, tag="otr")
                nc.sync.dma_start_transpose(otr[:], ocp[:])
                st["otr"] = otr

            def stage_E2(p, st):
                for c in range(CH):
                    nc.vector.tensor_copy(
                        out=od[:, p * CH + c, :, :],
                        in_=st["otr"][:, :, 16 * c:16 * (c + 1)])

            # modulo software pipeline over pairs: each emission step carries
            # L0(p), L1(p-1), L2(p-2), E(p-3) and trig for pair p+2 so every
            # in-order engine queue interleaves ~5 pairs' work.
            states = {}
            for p in range(min(2, npairs)):
                states[p] = {"trigs": [emit_phase_trig(p * CH + c)
                                       for c in range(CH)],
                             "m1s": [None] * CH, "P2s": [None] * CH,
                             "sqs": [None] * CH}
            for step in range(npairs + 4):
                if step < npairs:
                    stage_L(step, 0, states[step])
                if 0 <= step - 1 < npairs:
                    stage_L(step - 1, 1, states[step - 1])
                if 0 <= step - 2 < npairs:
                    stage_L(step - 2, 2, states[step - 2])
                if 0 <= step - 4 < npairs:
                    stage_E2(step - 4, states[step - 4])
                    del states[step - 4]
                if 0 <= step - 3 < npairs:
                    stage_E(step - 3, states[step - 3])
                if step + 2 < npairs:
                    states[step + 2] = {
                        "trigs": [emit_phase_trig((step + 2) * CH + c)
                                  for c in range(CH)],
                        "m1s": [None] * CH, "P2s": [None] * CH,
                        "sqs": [None] * CH}

            # ---- final store (chunked for overlap) ----------------------
            oflat = out_ap[:].rearrange("(p s) w -> p (s w)", p=128)
            tchunk = ntiles // 4
            for c in range(4):
                nc.sync.dma_start(
                    oflat[:, 64 * tchunk * c:64 * tchunk * (c + 1)],
                    od[:, tchunk * c:tchunk * (c + 1), :, :]
                    .rearrange("p a b c -> p (a b c)"))

    nc.compile()
    return nc


def _get_nc(bs, nu, lmap, dt_state_name=None):
    key = (bs, nu, tuple(lmap), dt_state_name or DT_STATE)
    if key not in _NC_CACHE:
        _NC_CACHE[key] = _build_nc(bs, nu, lmap, dt_state_name)
    return _NC_CACHE[key]


def _np_dt(name):
    import ml_dtypes
    return {"float32": np.float32, "float32r": np.float32,
            "float16": np.float16, "bfloat16": ml_dtypes.bfloat16}[name]


def kernel(x, weights, scaling):
    from concourse.bass_utils import run_bass_kernel_spmd

    x = np.ascontiguousarray(np.asarray(x, dtype=np.float32))
    B = x.shape[0]
    consts, nu, lmap = _host_tensors(weights, scaling,
                                     dt_state=_np_dt(DT_STATE))

    chunk = N_CORES * TS * 2
    Bp = ((B + chunk - 1) // chunk) * chunk
    if Bp != B:
        xp = np.zeros((Bp, x.shape[1]), dtype=np.float32)
        xp[:B] = x
        x = xp
    bs = Bp // N_CORES

    nc = _get_nc(bs, nu, lmap)
    xs = x.reshape(N_CORES, bs, x.shape[1])
    in_maps = [dict(consts, x=np.ascontiguousarray(xs[i]))
               for i in range(N_CORES)]
    res = run_bass_kernel_spmd(nc, in_maps, core_ids=list(range(N_CORES)))
    out = np.concatenate([r["out"] for r in res.results], axis=0)
    return out[:B]
